# revision 3
# baseline (speedup 1.0000x reference)
"""Trainium2 Bass kernel for the 2D-attention module (nn_Attention2D).

Reference computation (per batch element b):
    g_em   = img_fvec @ W1.T + b1                       # [HID]
    x_em   = conv3x3_same(patch_fmap, conv_w) + conv_b  # [HID, H, W]
    actv   = tanh(x_em + g_em[:, None, None])           # [HID, H, W]
    logits = W2 @ actv.reshape(HID, HW)                 # [1, HW]  (+b2, softmax-invariant)
    wts    = softmax(logits)                            # [1, HW]
    attn   = patch_fmap.reshape(C, HW) @ wts.T          # [C]

Sharding: pure data parallel, 8 images per core on 8 cores; weights replicated.

The conv (3.7 GFLOP/image) dominates: at bf16 it is PE-roofline bound
(~376us/core).  To beat that, the conv runs in fp8(e4m3) with the DoubleRow
perf mode (2 K-planes per instruction at 0.5 cycles/row) using a split-fp8
decomposition that preserves bf16-level accuracy:

    w*Sw = A + tau   (A = fp8(w*Sw), tau = fp8 residual)
    x*Sx = X + rho   (X = fp8(x*Sx), rho = fp8 residual)
    conv(w, x) * Sw*Sx ~= A@X + A@rho + tau@X       (drop tau@rho ~ 0.1%)

All three terms accumulate into one PSUM group; the 1/(Sw*Sx) scale folds into
the tanh activation's scale.  Quantization/padding is host-side numpy; the
device streams pre-padded fp8 tensors directly (no on-device pad/copy).

Per-core device program (channel-on-partition layout):
  - conv: per PSUM group [128 c_out, 392 pos]: 9 taps x 2 chunk-pairs x 3
    terms = 54 DoubleRow matmuls (rhs [128, 2, 14, 28] fp8 from the padded
    [128, 2, 30, 30] buffer).
  - tanh on ScalarE with per-partition bias = g_em[b] + b1 + conv_b and
    scale = 1/(Sw*Sx).
  - logits via M=1 matmuls contracted over c_out chunks (bf16 actv).
  - softmax on a single partition (DVE+ACT), normalized in SBUF.
  - partition-broadcast of normalized softmax weights via a DRAM bounce and a
    0-stride-partition SWDGE (gpsimd) DMA read-back.
  - weighted feature sum: DVE tensor_mul (patch_f32 * e) then ScalarE
    Identity-activation with accum_out -> attn[128, 1] per C_in chunk.
"""

import numpy as np
import ml_dtypes

import concourse.bass as bass
import concourse.bacc as bacc
import concourse.tile as tile
from concourse import mybir
from concourse.bass_utils import run_bass_kernel_spmd

# Problem shapes (hardcoded; kernel.py must be self-contained).
B = 64
C_IN = 512
HID = 512
H = W = 28
HW = H * W            # 784
N_CORES = 8
B_PER_CORE = B // N_CORES  # 8
KC = C_IN // 128      # 4 k chunks (contraction over c_in)
NPAIR = KC // 2       # 2 DoubleRow chunk-pairs
MC = HID // 128       # 4 m chunks (c_out partitions)
NH = 2                # spatial halves (14 rows x 28 cols = 392 <= 512 PSUM bank)
NHALF = HW // NH      # 392
ROWS_PER_HALF = H // NH  # 14

SX = 32.0             # patch fp8 scale (absmax*SX ~ 173 < 240)
SW = 2048.0           # conv_w fp8 scale (absmax*SW ~ 222 < 240)
SINV = 1.0 / (SX * SW)

FP32 = mybir.dt.float32
BF16 = mybir.dt.bfloat16
FP8 = mybir.dt.float8e4
NP_FP8 = ml_dtypes.float8_e4m3  # max 240, matches TRN float8e4

# conv term schedule: which (weight, x) plane products to accumulate.
# 'AX' = A@X (base), 'AR' = A@rho (x correction), 'TX' = tau@X (w correction).
TERMS = ("AX", "AR", "TX")


def build_bass():
    nc = bacc.Bacc(None)

    # Per-core inputs.
    patch_d = nc.dram_tensor("patch", [B_PER_CORE, C_IN, H, W], FP32,
                             kind="ExternalInput")
    xpad_d = nc.dram_tensor("xpad", [B_PER_CORE, NPAIR, 128, 2, H + 2, W + 2],
                            FP8, kind="ExternalInput")
    rpad_d = nc.dram_tensor("rpad", [B_PER_CORE, NPAIR, 128, 2, H + 2, W + 2],
                            FP8, kind="ExternalInput")
    awt_d = nc.dram_tensor("awt", [9, 128, NPAIR, 2, HID], FP8,
                           kind="ExternalInput")
    twt_d = nc.dram_tensor("twt", [9, 128, NPAIR, 2, HID], FP8,
                           kind="ExternalInput")
    imgT_d = nc.dram_tensor("imgT", [C_IN, B_PER_CORE], BF16,
                            kind="ExternalInput")
    w1t_d = nc.dram_tensor("w1t", [C_IN, HID], BF16, kind="ExternalInput")
    w2_d = nc.dram_tensor("w2", [HID], BF16, kind="ExternalInput")
    bsum_d = nc.dram_tensor("bsum", [HID], FP32, kind="ExternalInput")
    # Output laid out to match attn_sb exactly ([partition, k, b]) so the
    # final DMA is a single contiguous copy; the host transposes.
    out_d = nc.dram_tensor("out", [128, KC, B_PER_CORE], FP32,
                           kind="ExternalOutput")

    with tile.TileContext(nc) as tc:
        with (
            tc.tile_pool(name="wpool", bufs=1) as wpool,
            tc.tile_pool(name="pfpool", bufs=16) as pfpool,
            tc.tile_pool(name="padpool", bufs=12) as padpool,
            tc.tile_pool(name="actvpool", bufs=3) as actvpool,
            tc.tile_pool(name="spool", bufs=4) as spool,
            tc.tile_pool(name="scrpool", bufs=3) as scrpool,
            tc.tile_pool(name="ebspool", bufs=3) as ebspool,
            tc.tile_pool(name="dpool", bufs=4, space="DRAM") as dpool,
            tc.tile_pool(name="cpool", bufs=6, space="PSUM") as cpool,
            tc.tile_pool(name="lpool", bufs=1, space="PSUM") as lpool,
        ):
            # ---- Preload weights/constants ----
            # Small tensors first, then image-0 inputs, then conv weights
            # per tap so the first conv matmuls can start early.
            w1t_sb = wpool.tile([128, KC, HID], BF16)
            nc.sync.dma_start(
                out=w1t_sb, in_=w1t_d[:].rearrange("(k p) c -> p k c", p=128)
            )
            imgT_sb = wpool.tile([128, KC, B_PER_CORE], BF16)
            nc.sync.dma_start(
                out=imgT_sb, in_=imgT_d[:].rearrange("(k p) b -> p k b", p=128)
            )
            w2_sb = wpool.tile([128, MC], BF16)
            nc.sync.dma_start(
                out=w2_sb, in_=w2_d[:].rearrange("(k p) -> p k", p=128)
            )
            bsum_sb = wpool.tile([128, MC], FP32)
            nc.sync.dma_start(
                out=bsum_sb, in_=bsum_d[:].rearrange("(k p) -> p k", p=128)
            )
            awt_sb = wpool.tile([128, 9, NPAIR, 2, HID], FP8)
            twt_sb = wpool.tile([128, 9, NPAIR, 2, HID], FP8)
            # ---- g_em for all images: gbias[c_out, m, b] = W1@img + b1 + conv_b
            gbias_sb = wpool.tile([128, MC, B_PER_CORE], FP32)
            for m in range(MC):
                gps = cpool.tile([128, B_PER_CORE], FP32, tag="cps")
                for k in range(KC):
                    nc.tensor.matmul(
                        gps,
                        w1t_sb[:, k, m * 128:(m + 1) * 128],
                        imgT_sb[:, k, :],
                        start=(k == 0),
                        stop=(k == KC - 1),
                    )
                nc.scalar.activation(
                    out=gbias_sb[:, m, :],
                    in_=gps,
                    func=mybir.ActivationFunctionType.Identity,
                    bias=bsum_sb[:, m:m + 1],
                    scale=1.0,
                )

            # ---- Per-image pipeline ----
            state = {}  # image index -> tiles produced/needed per stage

            def emit_loads(b):
                pfs = []
                for k in range(KC):
                    pf = pfpool.tile([128, H, W], FP32, tag="pf")
                    nc.sync.dma_start(
                        out=pf, in_=patch_d[b, k * 128:(k + 1) * 128, :, :]
                    )
                    pfs.append(pf)
                xps, rps = [], []
                for pr in range(NPAIR):
                    xp = padpool.tile([128, 2, H + 2, W + 2], FP8, tag="xp")
                    nc.sync.dma_start(out=xp, in_=xpad_d[b, pr])
                    xps.append(xp)
                    rp = padpool.tile([128, 2, H + 2, W + 2], FP8, tag="rp")
                    nc.sync.dma_start(out=rp, in_=rpad_d[b, pr])
                    rps.append(rp)
                state[b] = {"pfs": pfs, "xps": xps, "rps": rps}

            def conv_planes(b, t, pr, m):
                """(lhsT, rhs) DoubleRow operands for one tap/pair/m-chunk."""
                st = state[b]
                dy, dx = t // 3, t % 3
                xs = st["xps"][pr]
                rs = st["rps"][pr]
                msl = slice(m * 128, (m + 1) * 128)
                out = []
                for term in TERMS:
                    wsb = twt_sb if term == "TX" else awt_sb
                    rhs_t = rs if term == "AR" else xs
                    out.append((wsb[:, t, pr, :, msl], rhs_t, dy, dx))
                return out

            def rhs_slice(rhs_t, h, dy, dx):
                r0 = h * ROWS_PER_HALF + dy
                return rhs_t[:, :, r0:r0 + ROWS_PER_HALF, dx:dx + W]

            def emit_conv(b, tap_streaming=False):
                actv = actvpool.tile([128, MC, HW], BF16, tag="actv")
                state[b]["actv"] = actv

                def tanh_out(cps, m, h):
                    nc.scalar.activation(
                        out=actv[:, m, h * NHALF:(h + 1) * NHALF],
                        in_=cps,
                        func=mybir.ActivationFunctionType.Tanh,
                        bias=gbias_sb[:, m, b:b + 1],
                        scale=SINV,
                    )

                n_per_group = 9 * NPAIR * len(TERMS)
                if tap_streaming:
                    # Image 0: weights are still streaming from HBM. Consume
                    # them tap-outer across 4 concurrent PSUM groups so the
                    # PE never waits for a late tap.
                    for h in range(NH):
                        cps_l = [cpool.tile([128, NHALF], FP32, tag="cps",
                                            name=f"cps0_{h}_{m}")
                                 for m in range(MC)]
                        idx = [0] * MC
                        for t in range(9):
                            for pr in range(NPAIR):
                                for m in range(MC):
                                    for lhsT, rhs_t, dy, dx in conv_planes(
                                            b, t, pr, m):
                                        nc.tensor.matmul(
                                            cps_l[m],
                                            lhsT,
                                            rhs_slice(rhs_t, h, dy, dx),
                                            start=(idx[m] == 0),
                                            stop=(idx[m] == n_per_group - 1),
                                            perf_mode=mybir.MatmulPerfMode.DoubleRow,
                                        )
                                        idx[m] += 1
                        for m in range(MC):
                            tanh_out(cps_l[m], m, h)
                    return

                for m in range(MC):
                    for h in range(NH):
                        cps = cpool.tile([128, NHALF], FP32, tag="cps")
                        idx = 0
                        for t in range(9):
                            for pr in range(NPAIR):
                                for lhsT, rhs_t, dy, dx in conv_planes(
                                        b, t, pr, m):
                                    nc.tensor.matmul(
                                        cps,
                                        lhsT,
                                        rhs_slice(rhs_t, h, dy, dx),
                                        start=(idx == 0),
                                        stop=(idx == n_per_group - 1),
                                        perf_mode=mybir.MatmulPerfMode.DoubleRow,
                                    )
                                    idx += 1
                        tanh_out(cps, m, h)

            def emit_finale1(b):
                """logits -> softmax -> normalized weights -> DRAM bounce."""
                actv = state[b]["actv"]
                lps = lpool.tile([1, NH, 512], FP32, tag="lps")
                for h in range(NH):
                    for m in range(MC):
                        nc.tensor.matmul(
                            lps[:, h, 0:NHALF],
                            w2_sb[:, m:m + 1],
                            actv[:, m, h * NHALF:(h + 1) * NHALF],
                            start=(m == 0),
                            stop=(m == MC - 1),
                        )
                # softmax on partition 0
                negmax = spool.tile([1, 1], FP32, tag="negmax")
                nc.vector.reduce_max(
                    out=negmax, in_=lps[:, :, 0:NHALF],
                    axis=mybir.AxisListType.XY, negate=True,
                )
                e_sb = spool.tile([1, HW], FP32, tag="e_sb")
                nc.scalar.activation(
                    out=e_sb.rearrange("p (h n) -> p h n", h=NH),
                    in_=lps[:, :, 0:NHALF],
                    func=mybir.ActivationFunctionType.Exp,
                    bias=negmax,
                    scale=1.0,
                )
                ssum = spool.tile([1, 1], FP32, tag="ssum")
                nc.vector.reduce_sum(out=ssum, in_=e_sb,
                                     axis=mybir.AxisListType.X)
                rsum = spool.tile([1, 1], FP32, tag="rsum")
                nc.vector.reciprocal(out=rsum, in_=ssum)
                en_sb = spool.tile([1, HW], FP32, tag="en_sb")
                nc.vector.tensor_scalar_mul(en_sb, e_sb, rsum)

                # stage the normalized weights in DRAM for partition-broadcast
                escr = dpool.tile([1, HW], FP32, tag="escr")
                nc.sync.dma_start(out=escr, in_=en_sb)
                state[b]["escr"] = escr

            def emit_finale2(b):
                """0-stride-partition SWDGE broadcast + weighted feature sum."""
                st = state.pop(b)
                escr = st["escr"]
                ebs = ebspool.tile([128, HW], FP32, tag="ebs")
                nc.gpsimd.dma_start(
                    out=ebs,
                    in_=bass.AP(tensor=escr.tensor, offset=escr.offset,
                                ap=[[0, 128], [1, HW]]),
                )
                for k in range(KC):
                    scr = scrpool.tile([128, HW], FP32, tag="scr")
                    nc.vector.tensor_tensor(
                        out=scr,
                        in0=st["pfs"][k].rearrange("p a b -> p (a b)"),
                        in1=ebs,
                        op=mybir.AluOpType.mult,
                    )
                    nc.scalar.activation(
                        out=scr,
                        in_=scr,
                        func=mybir.ActivationFunctionType.Identity,
                        accum_out=attn_sb[:, k, b:b + 1],
                    )

            attn_sb = wpool.tile([128, KC, B_PER_CORE], FP32)
            emit_loads(0)
            for t in range(9):
                nc.sync.dma_start(out=awt_sb[:, t], in_=awt_d[t])
                nc.sync.dma_start(out=twt_sb[:, t], in_=twt_d[t])
            for b in range(B_PER_CORE):
                if b + 1 < B_PER_CORE:
                    emit_loads(b + 1)
                emit_conv(b, tap_streaming=(b == 0))
                if b >= 1:
                    emit_finale1(b - 1)
                if b >= 2:
                    emit_finale2(b - 2)
            emit_finale2(B_PER_CORE - 2)
            emit_finale1(B_PER_CORE - 1)
            emit_finale2(B_PER_CORE - 1)

            nc.sync.dma_start(out=out_d[:], in_=attn_sb)

    nc.compile()
    return nc


_CACHED = {}


def get_bass():
    if "nc" not in _CACHED:
        _CACHED["nc"] = build_bass()
    return _CACHED["nc"]


def _pad_fp8(x8):
    """[B, C, H, W] fp8 -> [B, NPAIR, 128, 2, H+2, W+2] fp8, zero-padded."""
    b = x8.shape[0]
    out = np.zeros((b, C_IN, H + 2, W + 2), dtype=NP_FP8)
    out[:, :, 1:H + 1, 1:W + 1] = x8
    # c_in = pr*256 + j*128 + p  ->  [b, pr, p, j, h, w]
    out = out.reshape(b, NPAIR, 2, 128, H + 2, W + 2).transpose(0, 1, 3, 2, 4, 5)
    return np.ascontiguousarray(out)


def make_in_maps(img_fvec, patch_fmap, W1, b1, conv_w, conv_b, W2, b2):
    img_fvec = np.asarray(img_fvec, dtype=np.float32)
    patch_fmap = np.ascontiguousarray(np.asarray(patch_fmap, dtype=np.float32))
    W1 = np.asarray(W1, dtype=np.float32)
    b1 = np.asarray(b1, dtype=np.float32)
    conv_w = np.asarray(conv_w, dtype=np.float32)
    conv_b = np.asarray(conv_b, dtype=np.float32)
    W2 = np.asarray(W2, dtype=np.float32)
    # b2 shifts every logit equally; softmax is shift-invariant, so it drops out.

    w1t = np.ascontiguousarray(W1.T).astype(ml_dtypes.bfloat16)
    w2 = np.ascontiguousarray(W2[0]).astype(ml_dtypes.bfloat16)
    bsum = np.ascontiguousarray(b1 + conv_b).astype(np.float32)

    # split-fp8 conv weights: [HID, C_IN, 3, 3] -> [9, 128, NPAIR, 2, HID]
    ws = np.clip(conv_w * SW, -240.0, 240.0)
    A = ws.astype(NP_FP8)
    tau = np.clip(ws - A.astype(np.float32), -240.0, 240.0).astype(NP_FP8)

    def wt_layout(wq):
        arr = wq.transpose(2, 3, 1, 0).reshape(9, C_IN, HID)
        arr = arr.reshape(9, NPAIR, 2, 128, HID).transpose(0, 3, 1, 2, 4)
        return np.ascontiguousarray(arr)

    awt = wt_layout(A)
    twt = wt_layout(tau)

    # split-fp8 patch: X = fp8(x*SX), rho = fp8(x*SX - X), both padded
    xs = np.clip(patch_fmap * SX, -240.0, 240.0)
    X = xs.astype(NP_FP8)
    rho = np.clip(xs - X.astype(np.float32), -240.0, 240.0).astype(NP_FP8)

    in_maps = []
    for c in range(N_CORES):
        sl = slice(c * B_PER_CORE, (c + 1) * B_PER_CORE)
        imgT = np.ascontiguousarray(img_fvec[sl].T).astype(ml_dtypes.bfloat16)
        in_maps.append({
            "patch": np.ascontiguousarray(patch_fmap[sl]),
            "xpad": _pad_fp8(X[sl]),
            "rpad": _pad_fp8(rho[sl]),
            "awt": awt,
            "twt": twt,
            "imgT": imgT,
            "w1t": w1t,
            "w2": w2,
            "bsum": bsum,
        })
    return in_maps


def kernel(img_fvec, patch_fmap, W1, b1, conv_w, conv_b, W2, b2,
           trace=False, **run_kwargs):
    nc = get_bass()
    in_maps = make_in_maps(img_fvec, patch_fmap, W1, b1, conv_w, conv_b,
                           W2, b2)
    res = run_bass_kernel_spmd(nc, in_maps, core_ids=list(range(N_CORES)),
                               trace=trace, **run_kwargs)
    # per-core result is [p, k, b] -> [b, k*128+p]
    out = np.concatenate(
        [r["out"].transpose(2, 1, 0).reshape(B_PER_CORE, C_IN)
         for r in res.results], axis=0)
    if trace:
        kernel.last_results = res
    return out


# revision 8
# speedup vs baseline: 2.0096x; 2.0096x over previous
"""Trainium2 Bass kernel for the 2D-attention module (nn_Attention2D).

Reference computation (per batch element b):
    g_em   = img_fvec @ W1.T + b1                       # [HID]
    x_em   = conv3x3_same(patch_fmap, conv_w) + conv_b  # [HID, H, W]
    actv   = tanh(x_em + g_em[:, None, None])           # [HID, H, W]
    logits = W2 @ actv.reshape(HID, HW)                 # [1, HW]  (+b2, softmax-invariant)
    wts    = softmax(logits)                            # [1, HW]
    attn   = patch_fmap.reshape(C, HW) @ wts.T          # [C]

Sharding: pure data parallel, 8 images per core on 8 cores; weights replicated.

The conv (3.7 GFLOP/image) dominates and is PE-roofline bound at bf16
(~376us/core as 9 taps x 4 cin-chunks = 36 matmul planes per PSUM group).
This kernel reduces PE work 1.5x with 1D Winograd F(2,3) along the x axis:

    per output column pair: y = At [ (G w_x) * (Bt d_x) ]
    U1[i, ky]  = sum_kx G[i, kx] conv_w[:, :, ky, kx]        (host, bf16)
    V[0..3]    = column combos (d0-d2, d1+d2, d2-d1, d1-d3)  (DVE, bf16)
    M[i]       = sum_{ky, cin} U1[i, ky]^T V[i](rows ky:)    (PE: 12 planes
                 per (m, i) group of N=392 -> 192 matmuls/image vs 288)
    Z0 = M0+M1+M2, Z1 = M1-M2-M3                             (Scalar copy +
                 DVE adds, single-PSUM-operand ops only)
    actv[:, :, p::2] = tanh(Zp + g_em + b1 + conv_b)         (ACT, strided)

The input is host-split into even/odd padded columns (xe/xo) so every DVE
transform op is bf16/SBUF/stride-1 (fast mode), and the finale's weighted
feature sum reads the same xe/xo tiles (no separate fp32 patch DMA).

Remaining per-core program (as before): logits via M=1 matmuls, softmax on
one partition, 0-stride SWDGE partition-broadcast of the normalized weights
(now bf16, split even/odd cols), DVE mult + ACT accumulate for attn.
"""

import numpy as np
import ml_dtypes

import concourse.bass as bass
import concourse.bacc as bacc
import concourse.tile as tile
from concourse import mybir
from concourse.bass_utils import run_bass_kernel_spmd

# Problem shapes (hardcoded; kernel.py must be self-contained).
B = 64
C_IN = 512
HID = 512
H = W = 28
HW = H * W            # 784
N_CORES = 8
B_PER_CORE = B // N_CORES  # 8
KC = C_IN // 128      # 4 k chunks (contraction over c_in)
MC = HID // 128       # 4 m chunks (c_out partitions)
NH = 2                # halves of HW for logits matmuls (392 <= 512 PSUM bank)
NHALF = HW // NH      # 392
TC = W // 2           # 14 Winograd column tiles
NPOS = H * TC         # 392 = positions per output column-parity

FP32 = mybir.dt.float32
BF16 = mybir.dt.bfloat16


def build_bass():
    nc = bacc.Bacc(None)

    # Per-core inputs.  xe/xo: patch padded to 30x30 (bf16), split into even
    # (cols 0,2..28) / odd (cols 1,3..29) padded columns, channel chunks on
    # the partition dim.
    xe_d = nc.dram_tensor("xe", [B_PER_CORE, KC, 128, H + 2, 15], BF16,
                          kind="ExternalInput")
    xo_d = nc.dram_tensor("xo", [B_PER_CORE, KC, 128, H + 2, 15], BF16,
                          kind="ExternalInput")
    # U1[ik = i*3+ky]: x-transformed conv weights.
    u_d = nc.dram_tensor("u1", [12, 128, KC, HID], BF16, kind="ExternalInput")
    imgT_d = nc.dram_tensor("imgT", [C_IN, B_PER_CORE], BF16,
                            kind="ExternalInput")
    w1t_d = nc.dram_tensor("w1t", [C_IN, HID], BF16, kind="ExternalInput")
    w2_d = nc.dram_tensor("w2", [HID], BF16, kind="ExternalInput")
    bsum_d = nc.dram_tensor("bsum", [HID], FP32, kind="ExternalInput")
    # Output laid out to match attn_sb exactly ([partition, k, b]) so the
    # final DMA is a single contiguous copy; the host transposes.
    out_d = nc.dram_tensor("out", [128, KC, B_PER_CORE], FP32,
                           kind="ExternalOutput")

    with tile.TileContext(nc) as tc:
        with (
            tc.tile_pool(name="wpool", bufs=1) as wpool,
            tc.tile_pool(name="xpool", bufs=4) as xpool,
            tc.tile_pool(name="vpool", bufs=2) as vpool,
            tc.tile_pool(name="actvpool", bufs=3) as actvpool,
            tc.tile_pool(name="zpool", bufs=2) as zpool,
            tc.tile_pool(name="spool", bufs=2) as spool,
            tc.tile_pool(name="scrpool", bufs=3) as scrpool,
            tc.tile_pool(name="ebspool", bufs=3) as ebspool,
            tc.tile_pool(name="dpool", bufs=4, space="DRAM") as dpool,
            tc.tile_pool(name="cpool", bufs=6, space="PSUM") as cpool,
            tc.tile_pool(name="lpool", bufs=1, space="PSUM") as lpool,
        ):
            # ---- Preload weights/constants ----
            w1t_sb = wpool.tile([128, KC, HID], BF16)
            nc.sync.dma_start(
                out=w1t_sb, in_=w1t_d[:].rearrange("(k p) c -> p k c", p=128)
            )
            imgT_sb = wpool.tile([128, KC, B_PER_CORE], BF16)
            nc.sync.dma_start(
                out=imgT_sb, in_=imgT_d[:].rearrange("(k p) b -> p k b", p=128)
            )
            w2_sb = wpool.tile([128, MC], BF16)
            nc.sync.dma_start(
                out=w2_sb, in_=w2_d[:].rearrange("(k p) -> p k", p=128)
            )
            bsum_sb = wpool.tile([128, MC], FP32)
            nc.sync.dma_start(
                out=bsum_sb, in_=bsum_d[:].rearrange("(k p) -> p k", p=128)
            )
            u_sb = wpool.tile([128, 12, KC, HID], BF16)
            # ---- g_em for all images: gbias[c_out, m, b] = W1@img + b1 + conv_b
            gbias_sb = wpool.tile([128, MC, B_PER_CORE], FP32)
            for m in range(MC):
                gps = cpool.tile([128, B_PER_CORE], FP32, tag="cps")
                for k in range(KC):
                    nc.tensor.matmul(
                        gps,
                        w1t_sb[:, k, m * 128:(m + 1) * 128],
                        imgT_sb[:, k, :],
                        start=(k == 0),
                        stop=(k == KC - 1),
                    )
                nc.scalar.activation(
                    out=gbias_sb[:, m, :],
                    in_=gps,
                    func=mybir.ActivationFunctionType.Identity,
                    bias=bsum_sb[:, m:m + 1],
                    scale=1.0,
                )

            # ---- Per-image pipeline ----
            state = {}  # image index -> tiles produced/needed per stage

            def emit_loads(b):
                xe = xpool.tile([128, KC, H + 2, 15], BF16, tag="xe")
                xo = xpool.tile([128, KC, H + 2, 15], BF16, tag="xo")
                for k in range(KC):
                    nc.sync.dma_start(out=xe[:, k], in_=xe_d[b, k])
                    nc.sync.dma_start(out=xo[:, k], in_=xo_d[b, k])
                state[b] = {"xe": xe, "xo": xo}

            def emit_transform(b):
                """V[i] = x-dir Winograd combos, one batched DVE op per i."""
                st = state[b]
                xe, xo = st["xe"], st["xo"]
                d0 = xe[:, :, :, 0:14]
                d2 = xe[:, :, :, 1:15]
                d1 = xo[:, :, :, 0:14]
                d3 = xo[:, :, :, 1:15]
                v = vpool.tile([128, 4, KC, H + 2, TC], BF16, tag="v")
                for i, (a0, a1, op) in enumerate([
                    (d0, d2, mybir.AluOpType.subtract),
                    (d1, d2, mybir.AluOpType.add),
                    (d2, d1, mybir.AluOpType.subtract),
                    (d1, d3, mybir.AluOpType.subtract),
                ]):
                    nc.vector.tensor_tensor(out=v[:, i], in0=a0, in1=a1, op=op)
                st["v"] = v

            def emit_conv(b):
                st = state[b]
                v = st["v"]
                actv = actvpool.tile([128, MC, H, W], BF16, tag="actv")
                st["actv"] = actv
                for m in range(MC):
                    msl = slice(m * 128, (m + 1) * 128)
                    cps_l = []
                    for i in range(4):
                        cps = cpool.tile([128, NPOS], FP32, tag="cps")
                        idx = 0
                        for ky in range(3):
                            for k in range(KC):
                                nc.tensor.matmul(
                                    cps,
                                    u_sb[:, i * 3 + ky, k, msl],
                                    v[:, i, k, ky:ky + H, :],
                                    start=(idx == 0),
                                    stop=(idx == 3 * KC - 1),
                                )
                                idx += 1
                        cps_l.append(cps)
                    # output transform: Z0 = M0+M1+M2, Z1 = M1-M2-M3.
                    # Scalar copies M0/M1 out of PSUM so each DVE op reads at
                    # most one PSUM operand.
                    s0 = zpool.tile([128, NPOS], BF16, tag="s0")
                    nc.scalar.copy(out=s0, in_=cps_l[0])
                    s1 = zpool.tile([128, NPOS], BF16, tag="s1")
                    nc.scalar.copy(out=s1, in_=cps_l[1])
                    t01 = zpool.tile([128, NPOS], BF16, tag="t01")
                    nc.vector.tensor_tensor(out=t01, in0=s0, in1=cps_l[1],
                                            op=mybir.AluOpType.add)
                    z0 = zpool.tile([128, NPOS], BF16, tag="z0")
                    nc.vector.tensor_tensor(out=z0, in0=t01, in1=cps_l[2],
                                            op=mybir.AluOpType.add)
                    t12 = zpool.tile([128, NPOS], BF16, tag="t12")
                    nc.vector.tensor_tensor(out=t12, in0=s1, in1=cps_l[2],
                                            op=mybir.AluOpType.subtract)
                    z1 = zpool.tile([128, NPOS], BF16, tag="z1")
                    nc.vector.tensor_tensor(out=z1, in0=t12, in1=cps_l[3],
                                            op=mybir.AluOpType.subtract)
                    for p, z in ((0, z0), (1, z1)):
                        nc.scalar.activation(
                            out=actv[:, m, :, p::2],
                            in_=z.rearrange("p (a t) -> p a t", a=H),
                            func=mybir.ActivationFunctionType.Tanh,
                            bias=gbias_sb[:, m, b:b + 1],
                            scale=1.0,
                        )

            def emit_finale1(b):
                """logits -> softmax -> normalized weights -> DRAM bounce."""
                actv = state[b]["actv"]
                lps = lpool.tile([1, NH, 512], FP32, tag="lps")
                for h in range(NH):
                    for m in range(MC):
                        flat = actv[:, m].rearrange("p a b -> p (a b)")
                        nc.tensor.matmul(
                            lps[:, h, 0:NHALF],
                            w2_sb[:, m:m + 1],
                            flat[:, h * NHALF:(h + 1) * NHALF],
                            start=(m == 0),
                            stop=(m == MC - 1),
                        )
                # softmax on partition 0
                negmax = spool.tile([1, 1], FP32, tag="negmax")
                nc.vector.reduce_max(
                    out=negmax, in_=lps[:, :, 0:NHALF],
                    axis=mybir.AxisListType.XY, negate=True,
                )
                # exp writes straight into even/odd-column-major order
                # [q(parity), row, tc] so the DRAM bounce + partition
                # broadcast stay contiguous.
                e_sb = spool.tile([1, 2, H, TC], FP32, tag="e_sb")
                nc.scalar.activation(
                    out=e_sb.rearrange("p q (h r) t -> p h r t q", h=NH),
                    in_=lps[:, :, 0:NHALF].rearrange(
                        "p h (r t q) -> p h r t q", t=TC, q=2),
                    func=mybir.ActivationFunctionType.Exp,
                    bias=negmax,
                    scale=1.0,
                )
                ssum = spool.tile([1, 1], FP32, tag="ssum")
                nc.vector.reduce_sum(out=ssum,
                                     in_=e_sb.rearrange("p q h t -> p (q h t)"),
                                     axis=mybir.AxisListType.X)
                rsum = spool.tile([1, 1], FP32, tag="rsum")
                nc.vector.reciprocal(out=rsum, in_=ssum)
                en_sb = spool.tile([1, HW], BF16, tag="en_sb")
                nc.vector.tensor_scalar_mul(
                    en_sb, e_sb.rearrange("p q h t -> p (q h t)"), rsum)

                # stage the normalized weights in DRAM for partition-broadcast
                escr = dpool.tile([1, HW], BF16, tag="escr")
                nc.sync.dma_start(out=escr, in_=en_sb)
                state[b]["escr"] = escr

            def emit_finale2(b):
                """0-stride-partition SWDGE broadcast + weighted feature sum."""
                st = state.pop(b)
                escr = st["escr"]
                xe, xo = st["xe"], st["xo"]
                # ebs[:, 0] = weights at even orig cols, ebs[:, 1] = odd
                # (escr is already stored parity-major, so one contiguous
                # 0-partition-stride broadcast covers both).
                ebs = ebspool.tile([128, 2, H, TC], BF16, tag="ebs")
                nc.gpsimd.dma_start(
                    out=ebs,
                    in_=bass.AP(tensor=escr.tensor, offset=escr.offset,
                                ap=[[0, 128], [1, HW]]),
                )
                for k in range(KC):
                    scr = scrpool.tile([128, 2, H, TC], BF16, tag="scr")
                    # orig even cols 0,2..26 live in xo (padded odd cols),
                    # orig odd cols 1,3..27 in xe; rows 1..28 drop the pad.
                    nc.vector.tensor_tensor(
                        out=scr[:, 0], in0=xo[:, k, 1:H + 1, 0:14],
                        in1=ebs[:, 0], op=mybir.AluOpType.mult,
                    )
                    nc.vector.tensor_tensor(
                        out=scr[:, 1], in0=xe[:, k, 1:H + 1, 1:15],
                        in1=ebs[:, 1], op=mybir.AluOpType.mult,
                    )
                    nc.scalar.activation(
                        out=scr,
                        in_=scr,
                        func=mybir.ActivationFunctionType.Identity,
                        accum_out=attn_sb[:, k, b:b + 1],
                    )

            attn_sb = wpool.tile([128, KC, B_PER_CORE], FP32)
            emit_loads(0)
            for ik in range(12):
                nc.sync.dma_start(out=u_sb[:, ik], in_=u_d[ik])
            emit_transform(0)
            for b in range(B_PER_CORE):
                if b + 1 < B_PER_CORE:
                    emit_loads(b + 1)
                emit_conv(b)
                if b + 1 < B_PER_CORE:
                    emit_transform(b + 1)
                if b >= 1:
                    emit_finale1(b - 1)
                if b >= 2:
                    emit_finale2(b - 2)
            emit_finale2(B_PER_CORE - 2)
            emit_finale1(B_PER_CORE - 1)
            emit_finale2(B_PER_CORE - 1)

            nc.sync.dma_start(out=out_d[:], in_=attn_sb)

    nc.compile()
    return nc


_CACHED = {}


def get_bass():
    if "nc" not in _CACHED:
        _CACHED["nc"] = build_bass()
    return _CACHED["nc"]


G_MAT = np.array([[1, 0, 0], [0.5, 0.5, 0.5], [0.5, -0.5, 0.5], [0, 0, 1]],
                 np.float32)


def make_in_maps(img_fvec, patch_fmap, W1, b1, conv_w, conv_b, W2, b2):
    img_fvec = np.asarray(img_fvec, dtype=np.float32)
    patch_fmap = np.asarray(patch_fmap, dtype=np.float32)
    W1 = np.asarray(W1, dtype=np.float32)
    b1 = np.asarray(b1, dtype=np.float32)
    conv_w = np.asarray(conv_w, dtype=np.float32)
    conv_b = np.asarray(conv_b, dtype=np.float32)
    W2 = np.asarray(W2, dtype=np.float32)
    # b2 shifts every logit equally; softmax is shift-invariant, so it drops out.

    w1t = np.ascontiguousarray(W1.T).astype(ml_dtypes.bfloat16)
    w2 = np.ascontiguousarray(W2[0]).astype(ml_dtypes.bfloat16)
    bsum = np.ascontiguousarray(b1 + conv_b).astype(np.float32)

    # U1[i, ky] = sum_kx G[i, kx] w[:, :, ky, kx] -> [12, 128, KC, HID]
    u1 = np.einsum("ix,ocyx->iyco", G_MAT, conv_w)  # [4, 3, C_IN, HID]
    u1 = u1.reshape(12, KC, 128, HID).transpose(0, 2, 1, 3)
    u1 = np.ascontiguousarray(u1).astype(ml_dtypes.bfloat16)

    # padded bf16 patch, split into even/odd padded columns
    xpad = np.zeros((B, C_IN, H + 2, W + 2), dtype=ml_dtypes.bfloat16)
    xpad[:, :, 1:H + 1, 1:W + 1] = patch_fmap.astype(ml_dtypes.bfloat16)
    xe = np.ascontiguousarray(
        xpad[:, :, :, 0::2].reshape(B, KC, 128, H + 2, 15))
    xo = np.ascontiguousarray(
        xpad[:, :, :, 1::2].reshape(B, KC, 128, H + 2, 15))

    in_maps = []
    for c in range(N_CORES):
        sl = slice(c * B_PER_CORE, (c + 1) * B_PER_CORE)
        imgT = np.ascontiguousarray(img_fvec[sl].T).astype(ml_dtypes.bfloat16)
        in_maps.append({
            "xe": xe[sl],
            "xo": xo[sl],
            "u1": u1,
            "imgT": imgT,
            "w1t": w1t,
            "w2": w2,
            "bsum": bsum,
        })
    return in_maps


def kernel(img_fvec, patch_fmap, W1, b1, conv_w, conv_b, W2, b2,
           trace=False, **run_kwargs):
    nc = get_bass()
    in_maps = make_in_maps(img_fvec, patch_fmap, W1, b1, conv_w, conv_b,
                           W2, b2)
    res = run_bass_kernel_spmd(nc, in_maps, core_ids=list(range(N_CORES)),
                               trace=trace, **run_kwargs)
    # per-core result is [p, k, b] -> [b, k*128+p]
    out = np.concatenate(
        [r["out"].transpose(2, 1, 0).reshape(B_PER_CORE, C_IN)
         for r in res.results], axis=0)
    if trace:
        kernel.last_results = res
    return out


# revision 11
# speedup vs baseline: 2.0106x; 1.0005x over previous
"""Trainium2 Bass kernel for the 2D-attention module (nn_Attention2D).

Reference computation (per batch element b):
    g_em   = img_fvec @ W1.T + b1                       # [HID]
    x_em   = conv3x3_same(patch_fmap, conv_w) + conv_b  # [HID, H, W]
    actv   = tanh(x_em + g_em[:, None, None])           # [HID, H, W]
    logits = W2 @ actv.reshape(HID, HW)                 # [1, HW]  (+b2, softmax-invariant)
    wts    = softmax(logits)                            # [1, HW]
    attn   = patch_fmap.reshape(C, HW) @ wts.T          # [C]

Sharding: pure data parallel, 8 images per core on 8 cores; weights replicated.

The conv (3.7 GFLOP/image) dominates and is PE-roofline bound at bf16
(~376us/core as 9 taps x 4 cin-chunks = 36 matmul planes per PSUM group).
This kernel reduces PE work 1.5x with 1D Winograd F(2,3) along the x axis:

    per output column pair: y = At [ (G w_x) * (Bt d_x) ]
    U1[i, ky]  = sum_kx G[i, kx] conv_w[:, :, ky, kx]        (host, bf16)
    V[0..3]    = column combos (d0-d2, d1+d2, d2-d1, d1-d3)  (DVE, bf16)
    M[i]       = sum_{ky, cin} U1[i, ky]^T V[i](rows ky:)    (PE: 12 planes
                 per (m, i) group of N=392 -> 192 matmuls/image vs 288)
    Z0 = M0+M1+M2, Z1 = M1-M2-M3                             (Scalar copy +
                 DVE adds, single-PSUM-operand ops only)
    actv[:, :, p::2] = tanh(Zp + g_em + b1 + conv_b)         (ACT, strided)

The input is host-split into even/odd padded columns (xe/xo) so every DVE
transform op is bf16/SBUF/stride-1 (fast mode), and the finale's weighted
feature sum reads the same xe/xo tiles (no separate fp32 patch DMA).

Remaining per-core program (as before): logits via M=1 matmuls, softmax on
one partition, 0-stride SWDGE partition-broadcast of the normalized weights
(now bf16, split even/odd cols), DVE mult + ACT accumulate for attn.
"""

import numpy as np
import ml_dtypes

import concourse.bass as bass
import concourse.bacc as bacc
import concourse.tile as tile
from concourse import mybir
from concourse.bass_utils import run_bass_kernel_spmd

# Problem shapes (hardcoded; kernel.py must be self-contained).
B = 64
C_IN = 512
HID = 512
H = W = 28
HW = H * W            # 784
N_CORES = 8
B_PER_CORE = B // N_CORES  # 8
KC = C_IN // 128      # 4 k chunks (contraction over c_in)
MC = HID // 128       # 4 m chunks (c_out partitions)
NH = 2                # halves of HW for logits matmuls (392 <= 512 PSUM bank)
NHALF = HW // NH      # 392
TC = W // 2           # 14 Winograd column tiles
NPOS = H * TC         # 392 = positions per output column-parity

FP32 = mybir.dt.float32
BF16 = mybir.dt.bfloat16


def build_bass():
    nc = bacc.Bacc(None)

    # Per-core inputs.  xe/xo: patch padded to 30x30 (bf16), split into even
    # (cols 0,2..28) / odd (cols 1,3..29) padded columns, channel chunks on
    # the partition dim.
    xe_d = nc.dram_tensor("xe", [B_PER_CORE, KC, 128, H + 2, 15], BF16,
                          kind="ExternalInput")
    xo_d = nc.dram_tensor("xo", [B_PER_CORE, KC, 128, H + 2, 15], BF16,
                          kind="ExternalInput")
    # U1[ik = i*3+ky]: x-transformed conv weights.
    u_d = nc.dram_tensor("u1", [12, 128, KC, HID], BF16, kind="ExternalInput")
    imgT_d = nc.dram_tensor("imgT", [C_IN, B_PER_CORE], BF16,
                            kind="ExternalInput")
    w1t_d = nc.dram_tensor("w1t", [C_IN, HID], BF16, kind="ExternalInput")
    w2_d = nc.dram_tensor("w2", [HID], BF16, kind="ExternalInput")
    bsum_d = nc.dram_tensor("bsum", [HID], FP32, kind="ExternalInput")
    # Output laid out to match attn_sb exactly ([partition, k, b]) so the
    # final DMA is a single contiguous copy; the host transposes.
    out_d = nc.dram_tensor("out", [128, KC, B_PER_CORE], FP32,
                           kind="ExternalOutput")

    with tile.TileContext(nc) as tc:
        with (
            tc.tile_pool(name="wpool", bufs=1) as wpool,
            tc.tile_pool(name="xpool", bufs=4) as xpool,
            tc.tile_pool(name="vpool", bufs=2) as vpool,
            tc.tile_pool(name="actvpool", bufs=3) as actvpool,
            tc.tile_pool(name="zpool", bufs=2) as zpool,
            tc.tile_pool(name="spool", bufs=2) as spool,
            tc.tile_pool(name="scrpool", bufs=3) as scrpool,
            tc.tile_pool(name="ebspool", bufs=3) as ebspool,
            tc.tile_pool(name="dpool", bufs=4, space="DRAM") as dpool,
            tc.tile_pool(name="cpool", bufs=6, space="PSUM") as cpool,
            tc.tile_pool(name="lpool", bufs=1, space="PSUM") as lpool,
        ):
            # ---- Preload weights/constants ----
            w1t_sb = wpool.tile([128, KC, HID], BF16)
            nc.sync.dma_start(
                out=w1t_sb, in_=w1t_d[:].rearrange("(k p) c -> p k c", p=128)
            )
            imgT_sb = wpool.tile([128, KC, B_PER_CORE], BF16)
            nc.sync.dma_start(
                out=imgT_sb, in_=imgT_d[:].rearrange("(k p) b -> p k b", p=128)
            )
            w2_sb = wpool.tile([128, MC], BF16)
            nc.sync.dma_start(
                out=w2_sb, in_=w2_d[:].rearrange("(k p) -> p k", p=128)
            )
            bsum_sb = wpool.tile([128, MC], FP32)
            nc.sync.dma_start(
                out=bsum_sb, in_=bsum_d[:].rearrange("(k p) -> p k", p=128)
            )
            u_sb = wpool.tile([128, 12, KC, HID], BF16)
            # ---- g_em for all images: gbias[c_out, m, b] = W1@img + b1 + conv_b
            gbias_sb = wpool.tile([128, MC, B_PER_CORE], FP32)
            for m in range(MC):
                gps = cpool.tile([128, B_PER_CORE], FP32, tag="cps")
                for k in range(KC):
                    nc.tensor.matmul(
                        gps,
                        w1t_sb[:, k, m * 128:(m + 1) * 128],
                        imgT_sb[:, k, :],
                        start=(k == 0),
                        stop=(k == KC - 1),
                    )
                nc.scalar.activation(
                    out=gbias_sb[:, m, :],
                    in_=gps,
                    func=mybir.ActivationFunctionType.Identity,
                    bias=bsum_sb[:, m:m + 1],
                    scale=1.0,
                )

            # ---- Per-image pipeline ----
            state = {}  # image index -> tiles produced/needed per stage

            def emit_loads(b):
                xe = xpool.tile([128, KC, H + 2, 15], BF16, tag="xe")
                xo = xpool.tile([128, KC, H + 2, 15], BF16, tag="xo")
                for k in range(KC):
                    nc.sync.dma_start(out=xe[:, k], in_=xe_d[b, k])
                    nc.sync.dma_start(out=xo[:, k], in_=xo_d[b, k])
                state[b] = {"xe": xe, "xo": xo}

            def emit_transform(b):
                """V[i] = x-dir Winograd combos, one batched DVE op per i."""
                st = state[b]
                xe, xo = st["xe"], st["xo"]
                d0 = xe[:, :, :, 0:14]
                d2 = xe[:, :, :, 1:15]
                d1 = xo[:, :, :, 0:14]
                d3 = xo[:, :, :, 1:15]
                v = vpool.tile([128, 4, KC, H + 2, TC], BF16, tag="v")
                for i, (a0, a1, op) in enumerate([
                    (d0, d2, mybir.AluOpType.subtract),
                    (d1, d2, mybir.AluOpType.add),
                    (d2, d1, mybir.AluOpType.subtract),
                    (d1, d3, mybir.AluOpType.subtract),
                ]):
                    nc.vector.tensor_tensor(out=v[:, i], in0=a0, in1=a1, op=op)
                st["v"] = v

            def emit_conv(b):
                st = state[b]
                v = st["v"]
                actv = actvpool.tile([128, MC, H, W], BF16, tag="actv")
                st["actv"] = actv
                for m in range(MC):
                    msl = slice(m * 128, (m + 1) * 128)
                    cps_l = []
                    for i in range(4):
                        cps = cpool.tile([128, NPOS], FP32, tag="cps")
                        idx = 0
                        for ky in range(3):
                            for k in range(KC):
                                nc.tensor.matmul(
                                    cps,
                                    u_sb[:, i * 3 + ky, k, msl],
                                    v[:, i, k, ky:ky + H, :],
                                    start=(idx == 0),
                                    stop=(idx == 3 * KC - 1),
                                )
                                idx += 1
                        cps_l.append(cps)
                    # output transform: Z0 = M0+M1+M2, Z1 = M1-M2-M3.
                    # DVE may read at most one PSUM operand per op (walrus
                    # NCC_IBVF027), so M1 -- used by both chains -- goes
                    # through one Scalar copy.
                    s1 = zpool.tile([128, NPOS], BF16, tag="s1")
                    nc.scalar.copy(out=s1, in_=cps_l[1])
                    t01 = zpool.tile([128, NPOS], BF16, tag="t01")
                    nc.vector.tensor_tensor(out=t01, in0=s1, in1=cps_l[0],
                                            op=mybir.AluOpType.add)
                    z0 = zpool.tile([128, NPOS], BF16, tag="z0")
                    nc.vector.tensor_tensor(out=z0, in0=t01, in1=cps_l[2],
                                            op=mybir.AluOpType.add)
                    t12 = zpool.tile([128, NPOS], BF16, tag="t12")
                    nc.vector.tensor_tensor(out=t12, in0=s1, in1=cps_l[2],
                                            op=mybir.AluOpType.subtract)
                    z1 = zpool.tile([128, NPOS], BF16, tag="z1")
                    nc.vector.tensor_tensor(out=z1, in0=t12, in1=cps_l[3],
                                            op=mybir.AluOpType.subtract)
                    for p, z in ((0, z0), (1, z1)):
                        nc.scalar.activation(
                            out=actv[:, m, :, p::2],
                            in_=z.rearrange("p (a t) -> p a t", a=H),
                            func=mybir.ActivationFunctionType.Tanh,
                            bias=gbias_sb[:, m, b:b + 1],
                            scale=1.0,
                        )

            def emit_finale1(b):
                """logits -> softmax -> normalized weights -> DRAM bounce."""
                actv = state[b]["actv"]
                lps = lpool.tile([1, NH, 512], FP32, tag="lps")
                for h in range(NH):
                    for m in range(MC):
                        flat = actv[:, m].rearrange("p a b -> p (a b)")
                        nc.tensor.matmul(
                            lps[:, h, 0:NHALF],
                            w2_sb[:, m:m + 1],
                            flat[:, h * NHALF:(h + 1) * NHALF],
                            start=(m == 0),
                            stop=(m == MC - 1),
                        )
                # softmax on partition 0
                negmax = spool.tile([1, 1], FP32, tag="negmax")
                nc.vector.reduce_max(
                    out=negmax, in_=lps[:, :, 0:NHALF],
                    axis=mybir.AxisListType.XY, negate=True,
                )
                # exp writes straight into even/odd-column-major order
                # [q(parity), row, tc] so the DRAM bounce + partition
                # broadcast stay contiguous.
                e_sb = spool.tile([1, 2, H, TC], FP32, tag="e_sb")
                nc.scalar.activation(
                    out=e_sb.rearrange("p q (h r) t -> p h r t q", h=NH),
                    in_=lps[:, :, 0:NHALF].rearrange(
                        "p h (r t q) -> p h r t q", t=TC, q=2),
                    func=mybir.ActivationFunctionType.Exp,
                    bias=negmax,
                    scale=1.0,
                )
                ssum = spool.tile([1, 1], FP32, tag="ssum")
                nc.vector.reduce_sum(out=ssum,
                                     in_=e_sb.rearrange("p q h t -> p (q h t)"),
                                     axis=mybir.AxisListType.X)
                rsum = spool.tile([1, 1], FP32, tag="rsum")
                nc.vector.reciprocal(out=rsum, in_=ssum)
                en_sb = spool.tile([1, HW], BF16, tag="en_sb")
                nc.vector.tensor_scalar_mul(
                    en_sb, e_sb.rearrange("p q h t -> p (q h t)"), rsum)

                # stage the normalized weights in DRAM for partition-broadcast
                escr = dpool.tile([1, HW], BF16, tag="escr")
                nc.sync.dma_start(out=escr, in_=en_sb)
                state[b]["escr"] = escr

            def emit_finale2(b):
                """0-stride-partition SWDGE broadcast + weighted feature sum."""
                st = state.pop(b)
                escr = st["escr"]
                xe, xo = st["xe"], st["xo"]
                # ebs[:, 0] = weights at even orig cols, ebs[:, 1] = odd
                # (escr is already stored parity-major, so one contiguous
                # 0-partition-stride broadcast covers both).
                ebs = ebspool.tile([128, 2, H, TC], BF16, tag="ebs")
                nc.gpsimd.dma_start(
                    out=ebs,
                    in_=bass.AP(tensor=escr.tensor, offset=escr.offset,
                                ap=[[0, 128], [1, HW]]),
                )
                for k in range(KC):
                    scr = scrpool.tile([128, 2, H, TC], BF16, tag="scr")
                    # orig even cols 0,2..26 live in xo (padded odd cols),
                    # orig odd cols 1,3..27 in xe; rows 1..28 drop the pad.
                    nc.vector.tensor_tensor(
                        out=scr[:, 0], in0=xo[:, k, 1:H + 1, 0:14],
                        in1=ebs[:, 0], op=mybir.AluOpType.mult,
                    )
                    nc.vector.tensor_tensor(
                        out=scr[:, 1], in0=xe[:, k, 1:H + 1, 1:15],
                        in1=ebs[:, 1], op=mybir.AluOpType.mult,
                    )
                    nc.scalar.activation(
                        out=scr,
                        in_=scr,
                        func=mybir.ActivationFunctionType.Identity,
                        accum_out=attn_sb[:, k, b:b + 1],
                    )

            attn_sb = wpool.tile([128, KC, B_PER_CORE], FP32)
            emit_loads(0)
            for ik in range(12):
                nc.sync.dma_start(out=u_sb[:, ik], in_=u_d[ik])
            emit_transform(0)
            for b in range(B_PER_CORE):
                if b + 1 < B_PER_CORE:
                    emit_loads(b + 1)
                    emit_transform(b + 1)
                emit_conv(b)
                if b >= 1:
                    emit_finale1(b - 1)
                if b >= 2:
                    emit_finale2(b - 2)
            emit_finale2(B_PER_CORE - 2)
            emit_finale1(B_PER_CORE - 1)
            emit_finale2(B_PER_CORE - 1)

            nc.sync.dma_start(out=out_d[:], in_=attn_sb)

    nc.compile()
    return nc


_CACHED = {}


def get_bass():
    if "nc" not in _CACHED:
        _CACHED["nc"] = build_bass()
    return _CACHED["nc"]


G_MAT = np.array([[1, 0, 0], [0.5, 0.5, 0.5], [0.5, -0.5, 0.5], [0, 0, 1]],
                 np.float32)


def make_in_maps(img_fvec, patch_fmap, W1, b1, conv_w, conv_b, W2, b2):
    img_fvec = np.asarray(img_fvec, dtype=np.float32)
    patch_fmap = np.asarray(patch_fmap, dtype=np.float32)
    W1 = np.asarray(W1, dtype=np.float32)
    b1 = np.asarray(b1, dtype=np.float32)
    conv_w = np.asarray(conv_w, dtype=np.float32)
    conv_b = np.asarray(conv_b, dtype=np.float32)
    W2 = np.asarray(W2, dtype=np.float32)
    # b2 shifts every logit equally; softmax is shift-invariant, so it drops out.

    w1t = np.ascontiguousarray(W1.T).astype(ml_dtypes.bfloat16)
    w2 = np.ascontiguousarray(W2[0]).astype(ml_dtypes.bfloat16)
    bsum = np.ascontiguousarray(b1 + conv_b).astype(np.float32)

    # U1[i, ky] = sum_kx G[i, kx] w[:, :, ky, kx] -> [12, 128, KC, HID]
    u1 = np.einsum("ix,ocyx->iyco", G_MAT, conv_w)  # [4, 3, C_IN, HID]
    u1 = u1.reshape(12, KC, 128, HID).transpose(0, 2, 1, 3)
    u1 = np.ascontiguousarray(u1).astype(ml_dtypes.bfloat16)

    # padded bf16 patch, split into even/odd padded columns
    xpad = np.zeros((B, C_IN, H + 2, W + 2), dtype=ml_dtypes.bfloat16)
    xpad[:, :, 1:H + 1, 1:W + 1] = patch_fmap.astype(ml_dtypes.bfloat16)
    xe = np.ascontiguousarray(
        xpad[:, :, :, 0::2].reshape(B, KC, 128, H + 2, 15))
    xo = np.ascontiguousarray(
        xpad[:, :, :, 1::2].reshape(B, KC, 128, H + 2, 15))

    in_maps = []
    for c in range(N_CORES):
        sl = slice(c * B_PER_CORE, (c + 1) * B_PER_CORE)
        imgT = np.ascontiguousarray(img_fvec[sl].T).astype(ml_dtypes.bfloat16)
        in_maps.append({
            "xe": xe[sl],
            "xo": xo[sl],
            "u1": u1,
            "imgT": imgT,
            "w1t": w1t,
            "w2": w2,
            "bsum": bsum,
        })
    return in_maps


def kernel(img_fvec, patch_fmap, W1, b1, conv_w, conv_b, W2, b2,
           trace=False, **run_kwargs):
    nc = get_bass()
    in_maps = make_in_maps(img_fvec, patch_fmap, W1, b1, conv_w, conv_b,
                           W2, b2)
    res = run_bass_kernel_spmd(nc, in_maps, core_ids=list(range(N_CORES)),
                               trace=trace, **run_kwargs)
    # per-core result is [p, k, b] -> [b, k*128+p]
    out = np.concatenate(
        [r["out"].transpose(2, 1, 0).reshape(B_PER_CORE, C_IN)
         for r in res.results], axis=0)
    if trace:
        kernel.last_results = res
    return out


# revision 15
# speedup vs baseline: 2.0118x; 1.0006x over previous
"""Trainium2 Bass kernel for the 2D-attention module (nn_Attention2D).

Reference computation (per batch element b):
    g_em   = img_fvec @ W1.T + b1                       # [HID]
    x_em   = conv3x3_same(patch_fmap, conv_w) + conv_b  # [HID, H, W]
    actv   = tanh(x_em + g_em[:, None, None])           # [HID, H, W]
    logits = W2 @ actv.reshape(HID, HW)                 # [1, HW]  (+b2, softmax-invariant)
    wts    = softmax(logits)                            # [1, HW]
    attn   = patch_fmap.reshape(C, HW) @ wts.T          # [C]

Sharding: pure data parallel, 8 images per core on 8 cores; weights replicated.

The conv (3.7 GFLOP/image) dominates and is PE-roofline bound at bf16
(~376us/core as 9 taps x 4 cin-chunks = 36 matmul planes per PSUM group).
This kernel reduces PE work 1.5x with 1D Winograd F(2,3) along the x axis:

    per output column pair: y = At [ (G w_x) * (Bt d_x) ]
    U1[i, ky]  = sum_kx G[i, kx] conv_w[:, :, ky, kx]        (host, bf16)
    V[0..3]    = column combos (d0-d2, d1+d2, d2-d1, d1-d3)  (DVE, bf16)
    M[i]       = sum_{ky, cin} U1[i, ky]^T V[i](rows ky:)    (PE: 12 planes
                 per (m, i) group of N=392 -> 192 matmuls/image vs 288)
    Z0 = M0+M1+M2, Z1 = M1-M2-M3                             (Scalar copy +
                 DVE adds, single-PSUM-operand ops only)
    actv[:, :, p::2] = tanh(Zp + g_em + b1 + conv_b)         (ACT, strided)

The input is host-split into even/odd padded columns (xe/xo) so every DVE
transform op is bf16/SBUF/stride-1 (fast mode), and the finale's weighted
feature sum reads the same xe/xo tiles (no separate fp32 patch DMA).

Remaining per-core program (as before): logits via M=1 matmuls, softmax on
one partition, 0-stride SWDGE partition-broadcast of the normalized weights
(now bf16, split even/odd cols), DVE mult + ACT accumulate for attn.
"""

import numpy as np
import ml_dtypes

import concourse.bass as bass
import concourse.bacc as bacc
import concourse.tile as tile
from concourse import mybir
from concourse.bass_utils import run_bass_kernel_spmd

# Problem shapes (hardcoded; kernel.py must be self-contained).
B = 64
C_IN = 512
HID = 512
H = W = 28
HW = H * W            # 784
N_CORES = 8
B_PER_CORE = B // N_CORES  # 8
KC = C_IN // 128      # 4 k chunks (contraction over c_in)
MC = HID // 128       # 4 m chunks (c_out partitions)
NH = 2                # halves of HW for logits matmuls (392 <= 512 PSUM bank)
NHALF = HW // NH      # 392
TC = W // 2           # 14 Winograd column tiles
NPOS = H * TC         # 392 = positions per output column-parity

FP32 = mybir.dt.float32
BF16 = mybir.dt.bfloat16


def build_bass():
    nc = bacc.Bacc(None)

    # Per-core inputs.  xe/xo: patch padded to 30x30 (bf16), split into even
    # (cols 0,2..28) / odd (cols 1,3..29) padded columns, channel chunks on
    # the partition dim.
    xe_d = nc.dram_tensor("xe", [B_PER_CORE, KC, 128, H + 2, 15], BF16,
                          kind="ExternalInput")
    xo_d = nc.dram_tensor("xo", [B_PER_CORE, KC, 128, H + 2, 15], BF16,
                          kind="ExternalInput")
    # U1[ik = i*3+ky]: x-transformed conv weights.
    u_d = nc.dram_tensor("u1", [12, 128, KC, HID], BF16, kind="ExternalInput")
    imgT_d = nc.dram_tensor("imgT", [C_IN, B_PER_CORE], BF16,
                            kind="ExternalInput")
    w1t_d = nc.dram_tensor("w1t", [C_IN, HID], BF16, kind="ExternalInput")
    w2_d = nc.dram_tensor("w2", [HID], BF16, kind="ExternalInput")
    bsum_d = nc.dram_tensor("bsum", [HID], FP32, kind="ExternalInput")
    # Output laid out to match attn_sb exactly ([partition, k, b]) so the
    # final DMA is a single contiguous copy; the host transposes.
    out_d = nc.dram_tensor("out", [128, KC, B_PER_CORE], FP32,
                           kind="ExternalOutput")

    with tile.TileContext(nc) as tc:
        with (
            tc.tile_pool(name="wpool", bufs=1) as wpool,
            tc.tile_pool(name="xpool", bufs=4) as xpool,
            tc.tile_pool(name="vpool", bufs=2) as vpool,
            tc.tile_pool(name="actvpool", bufs=3) as actvpool,
            tc.tile_pool(name="zpool", bufs=2) as zpool,
            tc.tile_pool(name="spool", bufs=2) as spool,
            tc.tile_pool(name="scrpool", bufs=3) as scrpool,
            tc.tile_pool(name="ebspool", bufs=3) as ebspool,
            tc.tile_pool(name="cpool", bufs=5, space="PSUM") as cpool,
            tc.tile_pool(name="lpool", bufs=1, space="PSUM") as lpool,
            tc.tile_pool(name="bpool", bufs=1, space="PSUM") as bpool,
        ):
            # ---- Preload weights/constants ----
            w1t_sb = wpool.tile([128, KC, HID], BF16)
            nc.sync.dma_start(
                out=w1t_sb, in_=w1t_d[:].rearrange("(k p) c -> p k c", p=128)
            )
            imgT_sb = wpool.tile([128, KC, B_PER_CORE], BF16)
            nc.sync.dma_start(
                out=imgT_sb, in_=imgT_d[:].rearrange("(k p) b -> p k b", p=128)
            )
            w2_sb = wpool.tile([128, MC], BF16)
            nc.sync.dma_start(
                out=w2_sb, in_=w2_d[:].rearrange("(k p) -> p k", p=128)
            )
            bsum_sb = wpool.tile([128, MC], FP32)
            nc.sync.dma_start(
                out=bsum_sb, in_=bsum_d[:].rearrange("(k p) -> p k", p=128)
            )
            u_sb = wpool.tile([128, 12, KC, HID], BF16)
            ones_sb = wpool.tile([1, 128], BF16)
            nc.gpsimd.memset(ones_sb, 1.0)
            # ---- g_em for all images: gbias[c_out, m, b] = W1@img + b1 + conv_b
            gbias_sb = wpool.tile([128, MC, B_PER_CORE], FP32)
            for m in range(MC):
                gps = cpool.tile([128, B_PER_CORE], FP32, tag="cps")
                for k in range(KC):
                    nc.tensor.matmul(
                        gps,
                        w1t_sb[:, k, m * 128:(m + 1) * 128],
                        imgT_sb[:, k, :],
                        start=(k == 0),
                        stop=(k == KC - 1),
                    )
                nc.scalar.activation(
                    out=gbias_sb[:, m, :],
                    in_=gps,
                    func=mybir.ActivationFunctionType.Identity,
                    bias=bsum_sb[:, m:m + 1],
                    scale=1.0,
                )

            # ---- Per-image pipeline ----
            state = {}  # image index -> tiles produced/needed per stage

            def emit_loads(b):
                xe = xpool.tile([128, KC, H + 2, 15], BF16, tag="xe")
                xo = xpool.tile([128, KC, H + 2, 15], BF16, tag="xo")
                for k in range(KC):
                    nc.sync.dma_start(out=xe[:, k], in_=xe_d[b, k])
                    nc.sync.dma_start(out=xo[:, k], in_=xo_d[b, k])
                state[b] = {"xe": xe, "xo": xo}

            def emit_transform(b):
                """V[i] = x-dir Winograd combos, one batched DVE op per i."""
                st = state[b]
                xe, xo = st["xe"], st["xo"]
                d0 = xe[:, :, :, 0:14]
                d2 = xe[:, :, :, 1:15]
                d1 = xo[:, :, :, 0:14]
                d3 = xo[:, :, :, 1:15]
                v = vpool.tile([128, 4, KC, H + 2, TC], BF16, tag="v")
                for i, (a0, a1, op) in enumerate([
                    (d0, d2, mybir.AluOpType.subtract),
                    (d1, d2, mybir.AluOpType.add),
                    (d2, d1, mybir.AluOpType.subtract),
                    (d1, d3, mybir.AluOpType.subtract),
                ]):
                    nc.vector.tensor_tensor(out=v[:, i], in0=a0, in1=a1, op=op)
                st["v"] = v

            def emit_conv(b):
                st = state[b]
                v = st["v"]
                actv = actvpool.tile([128, MC, H, W], BF16, tag="actv")
                st["actv"] = actv
                for m in range(MC):
                    msl = slice(m * 128, (m + 1) * 128)
                    cps_l = []
                    for i in range(4):
                        cps = cpool.tile([128, NPOS], FP32, tag="cps")
                        idx = 0
                        for ky in range(3):
                            for k in range(KC):
                                nc.tensor.matmul(
                                    cps,
                                    u_sb[:, i * 3 + ky, k, msl],
                                    v[:, i, k, ky:ky + H, :],
                                    start=(idx == 0),
                                    stop=(idx == 3 * KC - 1),
                                )
                                idx += 1
                        cps_l.append(cps)
                    # output transform: Z0 = M0+M1+M2, Z1 = M1-M2-M3.
                    # DVE may read at most one PSUM operand per op (walrus
                    # NCC_IBVF027), so M1 -- used by both chains -- goes
                    # through one Scalar copy.
                    s1 = zpool.tile([128, NPOS], BF16, tag="s1")
                    nc.scalar.copy(out=s1, in_=cps_l[1])
                    t01 = zpool.tile([128, NPOS], BF16, tag="t01")
                    nc.vector.tensor_tensor(out=t01, in0=s1, in1=cps_l[0],
                                            op=mybir.AluOpType.add)
                    z0 = zpool.tile([128, NPOS], BF16, tag="z0")
                    nc.vector.tensor_tensor(out=z0, in0=t01, in1=cps_l[2],
                                            op=mybir.AluOpType.add)
                    t12 = zpool.tile([128, NPOS], BF16, tag="t12")
                    nc.vector.tensor_tensor(out=t12, in0=s1, in1=cps_l[2],
                                            op=mybir.AluOpType.subtract)
                    z1 = zpool.tile([128, NPOS], BF16, tag="z1")
                    nc.vector.tensor_tensor(out=z1, in0=t12, in1=cps_l[3],
                                            op=mybir.AluOpType.subtract)
                    for p, z in ((0, z0), (1, z1)):
                        nc.scalar.activation(
                            out=actv[:, m, :, p::2],
                            in_=z.rearrange("p (a t) -> p a t", a=H),
                            func=mybir.ActivationFunctionType.Tanh,
                            bias=gbias_sb[:, m, b:b + 1],
                            scale=1.0,
                        )

            def emit_finale1(b):
                """logits -> softmax -> normalized weights -> DRAM bounce."""
                actv = state[b]["actv"]
                lps = lpool.tile([1, NH, 512], FP32, tag="lps")
                for h in range(NH):
                    for m in range(MC):
                        flat = actv[:, m].rearrange("p a b -> p (a b)")
                        nc.tensor.matmul(
                            lps[:, h, 0:NHALF],
                            w2_sb[:, m:m + 1],
                            flat[:, h * NHALF:(h + 1) * NHALF],
                            start=(m == 0),
                            stop=(m == MC - 1),
                        )
                # softmax on partition 0
                negmax = spool.tile([1, 1], FP32, tag="negmax")
                nc.vector.reduce_max(
                    out=negmax, in_=lps[:, :, 0:NHALF],
                    axis=mybir.AxisListType.XY, negate=True,
                )
                # exp writes straight into even/odd-column-major order
                # [q(parity), row, tc] so the DRAM bounce + partition
                # broadcast stay contiguous.
                e_sb = spool.tile([1, 2, H, TC], FP32, tag="e_sb")
                nc.scalar.activation(
                    out=e_sb.rearrange("p q (h r) t -> p h r t q", h=NH),
                    in_=lps[:, :, 0:NHALF].rearrange(
                        "p h (r t q) -> p h r t q", t=TC, q=2),
                    func=mybir.ActivationFunctionType.Exp,
                    bias=negmax,
                    scale=1.0,
                )
                ssum = spool.tile([1, 1], FP32, tag="ssum")
                nc.vector.reduce_sum(out=ssum,
                                     in_=e_sb.rearrange("p q h t -> p (q h t)"),
                                     axis=mybir.AxisListType.X)
                rsum = spool.tile([1, 1], FP32, tag="rsum")
                nc.vector.reciprocal(out=rsum, in_=ssum)
                en_sb = spool.tile([1, HW], BF16, tag="en_sb")
                nc.vector.tensor_scalar_mul(
                    en_sb, e_sb.rearrange("p q h t -> p (q h t)"), rsum)
                state[b]["en"] = en_sb

            def emit_finale2(b):
                """Partition-broadcast of softmax weights via a K=1 matmul
                (ones^T @ e -> PSUM) + weighted feature sum."""
                st = state.pop(b)
                en_sb = st["en"]
                xe, xo = st["xe"], st["xo"]
                # ebs[:, 0] = weights at even orig cols, ebs[:, 1] = odd
                # (en_sb is stored parity-major, so the two broadcast halves
                # are exactly the two parities).
                ebs = ebspool.tile([128, 2, H, TC], BF16, tag="ebs")
                for par in range(2):
                    bps = bpool.tile([128, NPOS], FP32, tag="bps")
                    nc.tensor.matmul(
                        bps, ones_sb,
                        en_sb[:, par * NPOS:(par + 1) * NPOS],
                        start=True, stop=True,
                    )
                    nc.scalar.copy(out=ebs[:, par], in_=bps)
                for k in range(KC):
                    scr = scrpool.tile([128, 2, H, TC], BF16, tag="scr")
                    # orig even cols 0,2..26 live in xo (padded odd cols),
                    # orig odd cols 1,3..27 in xe; rows 1..28 drop the pad.
                    nc.vector.tensor_tensor(
                        out=scr[:, 0], in0=xo[:, k, 1:H + 1, 0:14],
                        in1=ebs[:, 0], op=mybir.AluOpType.mult,
                    )
                    nc.vector.tensor_tensor(
                        out=scr[:, 1], in0=xe[:, k, 1:H + 1, 1:15],
                        in1=ebs[:, 1], op=mybir.AluOpType.mult,
                    )
                    nc.scalar.activation(
                        out=scr,
                        in_=scr,
                        func=mybir.ActivationFunctionType.Identity,
                        accum_out=attn_sb[:, k, b:b + 1],
                    )

            attn_sb = wpool.tile([128, KC, B_PER_CORE], FP32)
            emit_loads(0)
            # stream U in image-0 consumption order (m-outer, then i, ky) in
            # per-m-chunk slices so the first conv group starts after ~3 small
            # DMAs instead of the whole 6.3 MB.
            for m in range(MC):
                msl = slice(m * 128, (m + 1) * 128)
                for ik in range(12):
                    nc.sync.dma_start(out=u_sb[:, ik, :, msl],
                                      in_=u_d[ik][:, :, msl])
            emit_transform(0)
            for b in range(B_PER_CORE):
                if b + 1 < B_PER_CORE:
                    emit_loads(b + 1)
                    emit_transform(b + 1)
                emit_conv(b)
                if b >= 1:
                    emit_finale1(b - 1)
                if b >= 2:
                    emit_finale2(b - 2)
            emit_finale2(B_PER_CORE - 2)
            emit_finale1(B_PER_CORE - 1)
            emit_finale2(B_PER_CORE - 1)

            nc.sync.dma_start(out=out_d[:], in_=attn_sb)

    nc.compile()
    return nc


_CACHED = {}


def get_bass():
    if "nc" not in _CACHED:
        _CACHED["nc"] = build_bass()
    return _CACHED["nc"]


G_MAT = np.array([[1, 0, 0], [0.5, 0.5, 0.5], [0.5, -0.5, 0.5], [0, 0, 1]],
                 np.float32)


def make_in_maps(img_fvec, patch_fmap, W1, b1, conv_w, conv_b, W2, b2):
    img_fvec = np.asarray(img_fvec, dtype=np.float32)
    patch_fmap = np.asarray(patch_fmap, dtype=np.float32)
    W1 = np.asarray(W1, dtype=np.float32)
    b1 = np.asarray(b1, dtype=np.float32)
    conv_w = np.asarray(conv_w, dtype=np.float32)
    conv_b = np.asarray(conv_b, dtype=np.float32)
    W2 = np.asarray(W2, dtype=np.float32)
    # b2 shifts every logit equally; softmax is shift-invariant, so it drops out.

    w1t = np.ascontiguousarray(W1.T).astype(ml_dtypes.bfloat16)
    w2 = np.ascontiguousarray(W2[0]).astype(ml_dtypes.bfloat16)
    bsum = np.ascontiguousarray(b1 + conv_b).astype(np.float32)

    # U1[i, ky] = sum_kx G[i, kx] w[:, :, ky, kx] -> [12, 128, KC, HID]
    u1 = np.einsum("ix,ocyx->iyco", G_MAT, conv_w)  # [4, 3, C_IN, HID]
    u1 = u1.reshape(12, KC, 128, HID).transpose(0, 2, 1, 3)
    u1 = np.ascontiguousarray(u1).astype(ml_dtypes.bfloat16)

    # padded bf16 patch, split into even/odd padded columns
    xpad = np.zeros((B, C_IN, H + 2, W + 2), dtype=ml_dtypes.bfloat16)
    xpad[:, :, 1:H + 1, 1:W + 1] = patch_fmap.astype(ml_dtypes.bfloat16)
    xe = np.ascontiguousarray(
        xpad[:, :, :, 0::2].reshape(B, KC, 128, H + 2, 15))
    xo = np.ascontiguousarray(
        xpad[:, :, :, 1::2].reshape(B, KC, 128, H + 2, 15))

    in_maps = []
    for c in range(N_CORES):
        sl = slice(c * B_PER_CORE, (c + 1) * B_PER_CORE)
        imgT = np.ascontiguousarray(img_fvec[sl].T).astype(ml_dtypes.bfloat16)
        in_maps.append({
            "xe": xe[sl],
            "xo": xo[sl],
            "u1": u1,
            "imgT": imgT,
            "w1t": w1t,
            "w2": w2,
            "bsum": bsum,
        })
    return in_maps


def kernel(img_fvec, patch_fmap, W1, b1, conv_w, conv_b, W2, b2,
           trace=False, **run_kwargs):
    nc = get_bass()
    in_maps = make_in_maps(img_fvec, patch_fmap, W1, b1, conv_w, conv_b,
                           W2, b2)
    res = run_bass_kernel_spmd(nc, in_maps, core_ids=list(range(N_CORES)),
                               trace=trace, **run_kwargs)
    # per-core result is [p, k, b] -> [b, k*128+p]
    out = np.concatenate(
        [r["out"].transpose(2, 1, 0).reshape(B_PER_CORE, C_IN)
         for r in res.results], axis=0)
    if trace:
        kernel.last_results = res
    return out
